# revision 35
# baseline (speedup 1.0000x reference)
"""Trainium2 Bass kernel for nn_ConditionedConvolution2D.

Reference computation:
    A  = P @ dense_w                      # [B, 3*3*C*C_OUT] per-sample conv kernels
    Wk = A.reshape(B, 3, 3, C, C_OUT)
    Y[b] = conv2d(X[b], Wk[b])            # SAME padding, stride 1, NHWC

Strategy (pure data parallel, 4 samples per core on 8 cores):
  - Host pre-lays X as a bf16 padded plane X_trip[b, ci, j, wp] =
    X[b, j, wp-1, ci] (zero at wp=0,129; 128 rows, pitch 130).  The device
    DMAs this into partitions 0-31 of the slab and builds the other two
    im2col groups on-chip with two shifted DVE copies per chunk
    (slab[32+ci, i] = slab[ci, i+1], slab[64+ci, i] = slab[ci, i+2]),
    giving the conv a ready-made stationary lhsT [96=(dw,ci), 128=w] per
    row while shipping only 1/3 of the im2col bytes over HBM.
  - Device computes the hypernetwork (per-sample kernels) with 96 small
    matmuls from a host-permuted dense_w so the weights land directly in
    [(dw,ci), (dh_rev,co)] streaming layout, then casts to bf16.
  - Conv: PSUM accumulators are full banks holding 16 output rows
    ([128 w, 16*32 (row,co)]).  For each row j (x row j = padded row j+1),
    a single matmul with moving operand up to [96, 96] writes the row
    chunks r = j-1, j, j+1 of the bank in one shot (per-element
    has_written gives accumulate-or-init per chunk).
  - DMA traffic is split over both HWDGE rings: slab loads on the SP
    (nc.sync) ring, dense_w/P loads and Y stores on the ACT (nc.scalar)
    ring, so input streaming and output draining overlap.
  - Completed 16-row banks are copied PSUM->SBUF with a cast to bf16
    (alternating DVE/ACT) into a per-sample staging buffer, DMA'd out in
    two 512KB transfers per sample to a [b, w, h*co] DRAM layout; the
    host transposes back to NHWC and upcasts.
"""

import os
import sys

sys.path.insert(0, "/opt/trn_rl_repo")

import numpy as np
import ml_dtypes

import concourse.bacc as bacc
import concourse.mybir as mybir
import concourse.tile as tile
from concourse.bass_utils import run_bass_kernel_spmd

B, H, W, C = 32, 128, 128, 32
P_DIM = 128
KH = KW = 3
C_OUT = 32
N_CORES = 8
BPC = B // N_CORES          # samples per core
W2 = W + 2                  # row pitch (data cols 0..128, one spare)
QK = KW * C                 # 96 contraction size (dw, ci)
G = KH * C_OUT              # 96 weight-stream columns per sample (dh_rev, co)
RPT = 16                    # output rows per PSUM tile (one full bank)
NT = H // RPT               # 8 PSUM tiles per sample
DW_CHUNKS = (40, 40, 16)    # dense_w load chunk sizes in g-blocks (sum 96);
                            # small last chunk shortens the phase-0 tail
SLAB_CHUNKS = (0, 44, 86, H)       # row boundaries of the slab load chunks

_NC_CACHE = {}


def _build_nc():
    f32 = mybir.dt.float32
    bf16 = mybir.dt.bfloat16
    nc = bacc.Bacc("TRN2", target_bir_lowering=False, debug=False,
                   num_devices=N_CORES)
    SC = SLAB_CHUNKS
    # chunk 0 of groups 0+1 (64 partitions), then chunks 1.. of group 0
    xa = nc.dram_tensor("xa", [BPC, 2 * C, SC[1] * W2], bf16,
                        kind="ExternalInput")
    xb = nc.dram_tensor("xb", [BPC, C, (H - SC[1]) * W2], bf16,
                        kind="ExternalInput")
    p_t = nc.dram_tensor("p_t", [P_DIM, BPC], bf16, kind="ExternalInput")
    dw_t = nc.dram_tensor("dw_t", [P_DIM, KH * KW * C * C_OUT], bf16,
                          kind="ExternalInput")
    y = nc.dram_tensor("y", [BPC, W, H * C_OUT], bf16, kind="ExternalOutput")

    with tile.TileContext(nc) as tc:
        with tc.tile_pool(name="const", bufs=1) as cpool, \
             tc.tile_pool(name="wsb", bufs=1) as wsb_pool, \
             tc.tile_pool(name="slab", bufs=4) as slab_pool, \
             tc.tile_pool(name="osb", bufs=2) as osb_pool:

            # ---- Phase 0: hypernetwork  Wk = P @ dense_w (permuted) ----
            # all INPUT loads go on the SP (sync) HWDGE ring in FIFO
            # priority order: p, dense_w, then the slab stream.  Output
            # stores ride the ACT ring so they never delay an input.
            p_sb = cpool.tile([P_DIM, BPC], bf16, name="p_sb", tag="p_sb")
            nc.sync.dma_start(out=p_sb[:], in_=p_t[:])
            dwsb = []
            g_off = 0
            for k, gch in enumerate(DW_CHUNKS):
                t = cpool.tile([P_DIM, gch * QK], bf16,
                               name=f"dwsb{k}", tag=f"dwsb{k}")
                nc.sync.dma_start(
                    out=t[:], in_=dw_t[:, g_off * QK:(g_off + gch) * QK])
                dwsb.append(t)
                g_off += gch

            # w_sb[q=(dw,ci), b*G + (2-dh)*C_OUT + co] (bf16 stream operand)
            w_sb = wsb_pool.tile([QK, BPC * G], bf16, name="w_sb", tag="w_sb")

            with tc.tile_pool(name="wps", bufs=len(DW_CHUNKS),
                              space="PSUM") as wps_pool:
                g_off = 0
                for k, gch in enumerate(DW_CHUNKS):
                    wps = wps_pool.tile([QK, gch * BPC], f32, name="wps",
                                        tag="wps")
                    for gk in range(gch):       # g = dh_rev*C_OUT + co
                        nc.tensor.matmul(
                            out=wps[:, gk * BPC:(gk + 1) * BPC],
                            lhsT=dwsb[k][:, gk * QK:(gk + 1) * QK],
                            rhs=p_sb[:],
                            start=True, stop=True,
                        )
                    # permute (g, b) -> (b, g) while casting f32 -> bf16
                    # (on ACT: DVE's strict-FIFO queue must stay free for
                    # the slab build copies, which gate the conv)
                    src = wps[:].rearrange("p (g b) -> p g b", b=BPC)
                    dst = w_sb[:].rearrange("p (b g) -> p g b", g=G)[
                        :, g_off:g_off + gch, :]
                    nc.scalar.copy(out=dst, in_=src)
                    g_off += gch

            # ---- Phase 1: per-sample conv ----
            # Output stores ride the same SP ring as the inputs, but their
            # trigger instructions are deferred by two samples in program
            # order so a not-yet-ready osb can never stall input prefetch.
            ydefer = [[] for _ in range(BPC)]
            with tc.tile_pool(name="acc", bufs=5, space="PSUM") as acc_pool:
                for b in range(BPC):
                    if b >= 2:
                        for dst, src in ydefer[b - 2]:
                            nc.sync.dma_start(out=dst, in_=src)
                    # chunked load of im2col group 0 (partitions 0..31),
                    # then build groups 1/2 with shifted on-chip copies
                    slab = slab_pool.tile([QK, H * W2], bf16, name="slab",
                                          tag="slab")
                    # chunk 0: groups 0+1 come from DRAM, build group 2
                    hi0 = SC[1] * W2
                    nc.sync.dma_start(out=slab[:2 * C, :hi0], in_=xa[b][:])
                    nc.vector.tensor_copy(
                        out=slab[2 * C:3 * C, :hi0 - 2],
                        in_=slab[:C, 2:hi0])
                    # later chunks: group 0 from DRAM, build groups 1+2
                    for ci in range(1, len(SC) - 1):
                        c0, c1 = SC[ci], SC[ci + 1]
                        lo, hi = c0 * W2, c1 * W2
                        nc.sync.dma_start(
                            out=slab[:C, lo:hi],
                            in_=xb[b][:, lo - hi0:hi - hi0])
                        nc.vector.tensor_copy(
                            out=slab[C:2 * C, lo:hi - 1],
                            in_=slab[:C, lo + 1:hi])
                        nc.vector.tensor_copy(
                            out=slab[2 * C:3 * C, lo:hi - 2],
                            in_=slab[:C, lo + 2:hi])

                    osb = osb_pool.tile([W, H * C_OUT], bf16, name="osb",
                                        tag="osb")
                    tiles = {}      # t -> psum AP [W, RPT*C_OUT]
                    for j in range(H):          # x row j = padded row j+1
                        hp = j + 1
                        lhsT = slab[:, j * W2: j * W2 + W]
                        # output rows touched by this X row, oldest first
                        rows = [r for r in (hp - 2, hp - 1, hp)
                                if 0 <= r < H]
                        # group into runs within one PSUM tile
                        groups = []
                        for r in rows:
                            t = r // RPT
                            if groups and groups[-1][0] == t:
                                groups[-1][1].append(r)
                            else:
                                groups.append((t, [r]))
                        for t, rs in groups:
                            if t not in tiles:
                                tiles[t] = acc_pool.tile(
                                    [W, RPT * C_OUT], f32, name="acc",
                                    tag="acc")
                            r_lo, r_hi = rs[0], rs[-1]
                            c_lo = r_lo % RPT
                            # dh for row r is hp-r; col block index is 2-dh
                            w_lo = 2 - (hp - r_lo)
                            first = ((r_lo % RPT == 0 and hp == r_lo)
                                     or (hp == 1 and r_lo == 0))
                            last = ((r_hi % RPT == RPT - 1
                                     and hp - r_hi == 2)
                                    or (hp == H and r_hi == H - 1))
                            nc.tensor.matmul(
                                out=tiles[t][:, c_lo * C_OUT:
                                             (c_lo + len(rs)) * C_OUT],
                                lhsT=lhsT,
                                rhs=w_sb[:, b * G + w_lo * C_OUT:
                                         b * G + (w_lo + len(rs)) * C_OUT],
                                start=first,
                                stop=last,
                                skip_group_check=True,
                            )
                        # tile t complete once row (t+1)*RPT-1 got its dh=2
                        t_done = None
                        if hp >= 2 and (hp - 2) % RPT == RPT - 1:
                            t_done = (hp - 2) // RPT
                        elif hp == H:
                            t_done = NT - 1     # row H-1 has no dh=2 input
                        if t_done is not None:
                            src = tiles.pop(t_done)
                            dst = osb[:, t_done * RPT * C_OUT:
                                      (t_done + 1) * RPT * C_OUT]
                            if b == BPC - 1 and t_done % 2 == 0:
                                nc.vector.tensor_copy(out=dst, in_=src[:])
                            else:
                                nc.scalar.copy(out=dst, in_=src[:])
                            # stream output out; finer splits for the last
                            # sample to shorten the kernel tail
                            step = 2 if b == BPC - 1 else 4
                            if (t_done + 1) % step == 0:
                                lo = (t_done + 1 - step) * RPT * C_OUT
                                hi = (t_done + 1) * RPT * C_OUT
                                ydefer[b].append(
                                    (y[b][:, lo:hi], osb[:, lo:hi]))
                for bb in range(max(0, BPC - 2), BPC):
                    for dst, src in ydefer[bb]:
                        nc.sync.dma_start(out=dst, in_=src)
    nc.finalize()
    return nc


def _get_nc():
    if "nc" not in _NC_CACHE:
        _NC_CACHE["nc"] = _build_nc()
    return _NC_CACHE["nc"]


def _prep_inputs(X, P, dense_w):
    bf16 = ml_dtypes.bfloat16
    Xb = np.ascontiguousarray(X.transpose(0, 3, 1, 2)).astype(bf16)  # [B,C,H,W]
    # group 0: g0[b, ci, j, wp] = X[b, j, wp-1, ci] (zero at wp=0, wp>=129)
    g0 = np.zeros((B, C, H, W2), dtype=bf16)
    g0[:, :, :, 1:W + 1] = Xb
    # group 1 (center) for chunk-0 rows: g1[.., j, wp] = X[b, j, wp, ci]
    S1 = SLAB_CHUNKS[1]
    g1 = np.zeros((B, C, S1, W2), dtype=bf16)
    g1[:, :, :, :W] = Xb[:, :, :S1]
    XA = np.concatenate([g0[:, :, :S1], g1], axis=1).reshape(B, 2 * C, -1)
    XB = np.ascontiguousarray(g0[:, :, S1:].reshape(B, C, -1))

    # dense_w columns j = ((dh*3+dw)*C+ci)*C_OUT+co -> (2-dh, co, dw, ci)
    dwp = np.ascontiguousarray(
        dense_w.reshape(P_DIM, KH, KW, C, C_OUT)[:, ::-1]
        .transpose(0, 1, 4, 2, 3)
        .reshape(P_DIM, -1)
    ).astype(bf16)

    in_maps = []
    for c in range(N_CORES):
        sl = slice(c * BPC, (c + 1) * BPC)
        in_maps.append({
            "xa": np.ascontiguousarray(XA[sl]),
            "xb": np.ascontiguousarray(XB[sl]),
            "p_t": np.ascontiguousarray(P[sl].T).astype(bf16),
            "dw_t": dwp,
        })
    return in_maps


def _run(X, P, dense_w, **spmd_kwargs):
    nc = _get_nc()
    in_maps = _prep_inputs(X, P, dense_w)
    res = run_bass_kernel_spmd(nc, in_maps, core_ids=list(range(N_CORES)),
                               **spmd_kwargs)
    outs = []
    for c in range(N_CORES):
        yv = res.results[c]["y"].astype(np.float32)
        yv = yv.reshape(BPC, W, H, C_OUT)
        outs.append(yv.transpose(0, 2, 1, 3))        # -> [b, h, w, co]
    Y = np.ascontiguousarray(np.concatenate(outs, axis=0), dtype=np.float32)
    return Y, res


def kernel(X, P, dense_w):
    Y, _ = _run(np.asarray(X), np.asarray(P), np.asarray(dense_w))
    return Y
